# revision 38
# baseline (speedup 1.0000x reference)
"""DifferentiableRAM (DRAW-style attention read) Trainium2 Bass kernel.

Reference computation (per batch b, channel c):
    gx = W*(p0+1)/2, gy = H*(p1+1)/2, sigma2 = exp(p2),
    delta = exp(p3)*(W-1)/(N-1), gamma = exp(p4)
    mu[i]  = g + delta*(i - N/2 - 0.5)                      i in [0,N)
    F[i,a] = exp(-(a-mu[i])^2 / (2 sigma2)) ;  Fn = F / (F.sum(a) + 1e-4)
    out[b,c] = gamma * Fy_n @ x[b,c] @ Fx_n^T                [N, N]

Strategy: pure data parallel over batch (B=32 -> 4 per core on 8 cores).
On-chip, both filterbanks are generated in *transposed* layout
T[a, i] = exp(-(a-mu_i)^2/(2s2)) (y and x side by side in one [128, 512]
tile per 128-row chunk) so both GEMMs contract along the partition axis
and the output lands in [n, m] row-major order:
    G1: FyxT[w, n] = sum_h x[h, w] * Ty[h, n]      (lhsT = x chunk)
    G2: raw[n, m]  = sum_w FyxT[w, n] * Tx[w, m]   (lhsT = FyxT chunk)
    out[n, m] = raw[n, m] * (gamma * invy[n]) * invx[m]
GEMMs run in bf16 (full PE rate; fp32 matmul is 1/4 rate, fp32r does not
survive walrus codegen).  Normalizers invy/invx = 1/(colsum + 1e-4) stay
fp32, computed as exp(-ln(colsum + 1e-4)) on ScalarE (ln+exp share one
LUT set) and applied to the fp32 PSUM of G2, so the final scaling is
full precision.
"""

import numpy as np
from contextlib import ExitStack

import concourse.tile as tile
from concourse import bacc, mybir
from concourse.bass_utils import run_bass_kernel_spmd

F32 = mybir.dt.float32
BF16 = mybir.dt.bfloat16
ALU = mybir.AluOpType
ACTF = mybir.ActivationFunctionType

B, C, H, W = 32, 3, 512, 512
N = 256
NCORES = 8
BL = B // NCORES  # batches per core
KC = 4            # 128-row chunks of the 512-long axis
SMALL = 1e-4
DELTA_SCALE = (max(W, H) - 1) / (N - 1.0)


def _kernel_body(tc):
    nc = tc.nc
    x_d = nc.dram_tensor("x", [BL, C, H, W], F32, kind="ExternalInput").ap()
    p_d = nc.dram_tensor("p", [BL, 5], F32, kind="ExternalInput").ap()
    o_d = nc.dram_tensor("out", [BL, C, N, N], F32, kind="ExternalOutput").ap()

    with ExitStack() as ctx:
        consts = ctx.enter_context(tc.tile_pool(name="consts", bufs=1))
        params = ctx.enter_context(tc.tile_pool(name="params", bufs=1))
        xf32p = ctx.enter_context(tc.tile_pool(name="xf32p", bufs=3))
        xbfp = ctx.enter_context(tc.tile_pool(name="xbfp", bufs=3))
        tban = ctx.enter_context(tc.tile_pool(name="tban", bufs=10))
        bcp = ctx.enter_context(tc.tile_pool(name="bcp", bufs=2))
        dtmp = ctx.enter_context(tc.tile_pool(name="dtmp", bufs=3))
        sqtmp = ctx.enter_context(tc.tile_pool(name="sqtmp", bufs=3))
        fyxp = ctx.enter_context(tc.tile_pool(name="fyxp", bufs=5))
        outp = ctx.enter_context(tc.tile_pool(name="outp", bufs=4))
        rows = ctx.enter_context(tc.tile_pool(name="rows", bufs=4))
        colp = ctx.enter_context(tc.tile_pool(name="colp", bufs=2))
        invp = ctx.enter_context(tc.tile_pool(name="invp", bufs=2))
        # PSUM: 8 banks total — ps1 2 + ps2 2 + pscs 1 + pscol 1 + psbc 1
        # + psinvx 1
        ps1 = ctx.enter_context(tc.tile_pool(name="ps1", bufs=2, space="PSUM"))
        ps2 = ctx.enter_context(tc.tile_pool(name="ps2", bufs=2, space="PSUM"))
        pscs = ctx.enter_context(tc.tile_pool(name="pscs", bufs=1, space="PSUM"))
        pscol = ctx.enter_context(tc.tile_pool(name="pscol", bufs=1, space="PSUM"))
        psbc = ctx.enter_context(tc.tile_pool(name="psbc", bufs=1, space="PSUM"))
        psinvx = ctx.enter_context(tc.tile_pool(name="psinvx", bufs=1, space="PSUM"))

        # ---- constants -------------------------------------------------
        a_iota = consts.tile([128, 1], F32)  # partition index 0..127
        nc.gpsimd.iota(a_iota, pattern=[[0, 1]], base=0, channel_multiplier=1,
                       allow_small_or_imprecise_dtypes=True)
        iota4 = consts.tile([BL, 2 * N], F32)  # 0..255 twice, on BL partitions
        nc.gpsimd.iota(iota4, pattern=[[0, 2], [1, N]], base=0,
                       channel_multiplier=0, allow_small_or_imprecise_dtypes=True)
        ones_k = consts.tile([128, 1], BF16)  # colsum lhsT
        nc.vector.memset(ones_k, 1.0)
        one1 = consts.tile([1, 1], F32)       # row->col rhs
        nc.vector.memset(one1, 1.0)
        ones_r = consts.tile([1, 128], F32)   # broadcast lhsT (1 -> 128 parts)
        nc.vector.memset(ones_r, 1.0)
        small1 = consts.tile([1, 1], F32)     # filterbank-normalizer epsilon
        nc.vector.memset(small1, SMALL)

        # ---- per-batch attention params (partition = batch) ------------
        pt = params.tile([BL, 5], F32)
        nc.sync.dma_start(out=pt, in_=p_d)
        E = params.tile([BL, 3], F32)  # [sigma2, exp(p3), gamma]
        nc.scalar.activation(E, pt[:, 2:5], ACTF.Exp)
        delta = params.tile([BL, 1], F32)
        nc.vector.tensor_scalar(delta, E[:, 1:2], DELTA_SCALE, None, ALU.mult)
        g2 = params.tile([BL, 2], F32)  # [gx, gy]
        nc.vector.tensor_scalar(g2, pt[:, 0:2], W / 2.0, W / 2.0, ALU.mult, ALU.add)
        cyx = params.tile([BL, 2], F32)  # g - (N/2+0.5)*delta ; [:,0]=y uses gy
        nc.vector.scalar_tensor_tensor(cyx[:, 0:1], delta, -(N / 2.0 + 0.5),
                                       g2[:, 1:2], ALU.mult, ALU.add)
        nc.vector.scalar_tensor_tensor(cyx[:, 1:2], delta, -(N / 2.0 + 0.5),
                                       g2[:, 0:1], ALU.mult, ALU.add)
        # per-batch row: [mu_y (N) | mu_x (N) | nhs | gamma]
        M4 = params.tile([BL, 2 * N + 2], F32)
        nc.vector.tensor_scalar(M4[:, 0:N], iota4[:, 0:N], delta, cyx[:, 0:1],
                                ALU.mult, ALU.add)
        nc.vector.tensor_scalar(M4[:, N:2 * N], iota4[:, N:2 * N], delta,
                                cyx[:, 1:2], ALU.mult, ALU.add)
        nc.vector.reciprocal(M4[:, 2 * N:2 * N + 1], E[:, 0:1])
        nc.vector.tensor_scalar(M4[:, 2 * N:2 * N + 1], M4[:, 2 * N:2 * N + 1],
                                -0.5, None, ALU.mult)
        nc.vector.tensor_copy(M4[:, 2 * N + 1:2 * N + 2], E[:, 2:3])

        # all batches' rows flattened onto partition 0 (one sbuf->sbuf DMA)
        RW = 2 * N + 2
        # stage + out-store DMAs go via the idle Pool engine's SWDGE queue:
        # their triggers wait on compute, and on the SP HWDGE ring such a
        # wait would stall the x-load triggers queued behind it.
        stage = params.tile([1, BL * RW], F32)
        for sb_ in range(BL):
            nc.gpsimd.dma_start(out=stage[:, sb_ * RW:(sb_ + 1) * RW],
                                in_=M4[sb_:sb_ + 1, :])

        for b in range(BL):
            # broadcast batch b's mu row + [nhs, gamma] to all 128 partitions
            r0 = b * RW
            ps_bc = psbc.tile([128, 2 * N], F32)
            nc.tensor.matmul(ps_bc, ones_r, stage[:, r0:r0 + 2 * N],
                             start=True, stop=True)
            bcmu = bcp.tile([128, 2 * N], F32)
            nc.scalar.copy(bcmu, ps_bc)
            ps_nhs = pscol.tile([128, 2], F32, tag="pcol")
            nc.tensor.matmul(ps_nhs, ones_r,
                             stage[:, r0 + 2 * N:r0 + 2 * N + 2],
                             start=True, stop=True)
            nhs_col = colp.tile([128, 2], F32)  # [:,0]=nhs  [:,1]=gamma
            nc.vector.tensor_copy(nhs_col, ps_nhs)

            # ---- filterbanks: Ty|Tx fused per chunk, unnormalized ------
            T = []
            invy_col = colp.tile([128, 2], F32)  # 1/(colsum_y+eps), n-major
            cs_ps = pscs.tile([1, 2 * N], F32)
            for k in range(KC):
                d_t = dtmp.tile([128, 2 * N], F32)
                # (mu - a_part) - 128k  (sign irrelevant after square)
                nc.vector.tensor_scalar(d_t, bcmu, a_iota, float(128 * k),
                                        ALU.subtract, ALU.subtract)
                sq_t = sqtmp.tile([128, 2 * N], F32)
                nc.scalar.activation(sq_t, d_t, ACTF.Square)
                T_t = tban.tile([128, 2 * N], BF16)
                nc.scalar.activation(T_t, sq_t, ACTF.Exp, scale=nhs_col[:, 0:1])
                T.append(T_t)
                nc.tensor.matmul(cs_ps, ones_k, T_t,
                                 start=(k == 0), stop=(k == KC - 1))
            # 1/(colsum + SMALL) = exp(-ln(colsum + SMALL)), both axes at once
            lnrow = rows.tile([1, 2 * N], F32)
            nc.scalar.activation(lnrow, cs_ps, ACTF.Ln, bias=small1[:, :])
            invrow = rows.tile([1, 2 * N], F32)
            nc.scalar.activation(invrow, lnrow, ACTF.Exp, scale=-1.0)
            # y-normalizer to column layout (n on partitions), * gamma
            for j in range(2):
                pcol = pscol.tile([128, 1], F32)
                nc.tensor.matmul(pcol, invrow[:, j * 128:(j + 1) * 128],
                                 one1, start=True, stop=True)
                nc.vector.tensor_scalar(invy_col[:, j:j + 1], pcol,
                                        nhs_col[:, 1:2], None, ALU.mult)
            # x-normalizer broadcast across partitions (m on free axis)
            invx_ps = psinvx.tile([128, N], F32)
            nc.tensor.matmul(invx_ps, ones_r, invrow[:, N:2 * N],
                             start=True, stop=True)
            invx_bc = invp.tile([128, N], F32)
            nc.scalar.copy(invx_bc, invx_ps)

            # ---- glimpse read: two chained GEMMs per channel -----------
            for c in range(C):
                xf = xf32p.tile([128, KC, W], F32)
                xt = xbfp.tile([128, KC, W], BF16)
                xsrc = x_d[b, c].rearrange("(hc p) w -> p hc w", p=128)
                for hc in range(KC):  # chunked so G1 starts as rows land
                    nc.sync.dma_start(out=xf[:, hc], in_=xsrc[:, hc])
                    nc.vector.tensor_copy(xt[:, hc], xf[:, hc])
                fyx = []
                for j in range(2):  # wc pairs
                    p1 = ps1.tile([128, 2 * N], F32)
                    for half in range(2):
                        wc = 2 * j + half
                        for hc in range(KC):
                            nc.tensor.matmul(
                                p1[:, half * N:(half + 1) * N],
                                xt[:, hc, wc * 128:(wc + 1) * 128],
                                T[hc][:, 0:N],
                                start=(hc == 0), stop=(hc == KC - 1))
                    f_t = fyxp.tile([128, 2 * N], BF16)
                    if j == 0:
                        nc.vector.tensor_copy(f_t, p1)
                    else:
                        nc.scalar.copy(f_t, p1)
                    fyx.append(f_t)
                ot = outp.tile([128, 2, N], F32)
                for nch in range(2):
                    p2 = ps2.tile([128, N], F32)
                    for wc in range(KC):
                        nc.tensor.matmul(
                            p2,
                            fyx[wc // 2][:, (wc % 2) * N + nch * 128:
                                         (wc % 2) * N + (nch + 1) * 128],
                            T[wc][:, N:2 * N],
                            start=(wc == 0), stop=(wc == KC - 1))
                    nc.vector.scalar_tensor_tensor(ot[:, nch, :], p2,
                                                   invy_col[:, nch:nch + 1],
                                                   invx_bc, ALU.mult, ALU.mult)
                nc.gpsimd.dma_start(
                    out=o_d[b, c].rearrange("(nch p) m -> p nch m", p=128), in_=ot)


_NC_CACHE = None


def _build():
    global _NC_CACHE
    if _NC_CACHE is None:
        nc = bacc.Bacc("TRN2", target_bir_lowering=False, debug=False,
                       enable_asserts=False, num_devices=NCORES)
        with tile.TileContext(nc) as tc:
            _kernel_body(tc)
        # Steer bacc's greedy ACT table-set choice to the one set that has
        # Exp+Ln+Square+Copy+Identity, else every per-batch Ln costs two
        # ~2.7us table reloads. Only the selection input is patched — set
        # ids and on-device table contents are untouched.
        ours = {ACTF.Exp, ACTF.Ln, ACTF.Square, ACTF.Copy, ACTF.Identity}
        keep = "natural_log_exp_and_others"
        orig = bacc.get_activation_tables

        def steered(arch):
            return {k: (v if k == keep else set(v) - ours)
                    for k, v in orig(arch).items()}

        bacc.get_activation_tables = steered
        try:
            nc.compile()
        finally:
            bacc.get_activation_tables = orig
        _NC_CACHE = nc
    return _NC_CACHE


def _run(x, p, trace=False, **kw):
    nc = _build()
    x = np.ascontiguousarray(x, dtype=np.float32)
    p = np.ascontiguousarray(p, dtype=np.float32)
    assert x.shape == (B, C, H, W) and p.shape == (B, 5), (x.shape, p.shape)
    in_maps = [
        {"x": x[i * BL:(i + 1) * BL], "p": p[i * BL:(i + 1) * BL]}
        for i in range(NCORES)
    ]
    res = run_bass_kernel_spmd(nc, in_maps, list(range(NCORES)), trace=trace, **kw)
    out = np.concatenate([res.results[i]["out"] for i in range(NCORES)], axis=0)
    return out, res


def kernel(x, p):
    out, _ = _run(x, p)
    return out


# revision 40
# speedup vs baseline: 1.0178x; 1.0178x over previous
"""DifferentiableRAM (DRAW-style attention read) Trainium2 Bass kernel.

Reference computation (per batch b, channel c):
    gx = W*(p0+1)/2, gy = H*(p1+1)/2, sigma2 = exp(p2),
    delta = exp(p3)*(W-1)/(N-1), gamma = exp(p4)
    mu[i]  = g + delta*(i - N/2 - 0.5)                      i in [0,N)
    F[i,a] = exp(-(a-mu[i])^2 / (2 sigma2)) ;  Fn = F / (F.sum(a) + 1e-4)
    out[b,c] = gamma * Fy_n @ x[b,c] @ Fx_n^T                [N, N]

Strategy: pure data parallel over batch (B=32 -> 4 per core on 8 cores).
On-chip, both filterbanks are generated in *transposed* layout
T[a, i] = exp(-(a-mu_i)^2/(2s2)) (y and x side by side in one [128, 512]
tile per 128-row chunk) so both GEMMs contract along the partition axis
and the output lands in [n, m] row-major order:
    G1: FyxT[w, n] = sum_h x[h, w] * Ty[h, n]      (lhsT = x chunk)
    G2: raw[n, m]  = sum_w FyxT[w, n] * Tx[w, m]   (lhsT = FyxT chunk)
    out[n, m] = raw[n, m] * (gamma * invy[n]) * invx[m]
GEMMs run in bf16 (full PE rate; fp32 matmul is 1/4 rate, fp32r does not
survive walrus codegen).  Normalizers invy/invx = 1/(colsum + 1e-4) stay
fp32, computed as exp(-ln(colsum + 1e-4)) on ScalarE (ln+exp share one
LUT set) and applied to the fp32 PSUM of G2, so the final scaling is
full precision.
"""

import numpy as np
from contextlib import ExitStack

import concourse.tile as tile
from concourse import bacc, mybir
from concourse.bass_utils import run_bass_kernel_spmd

F32 = mybir.dt.float32
BF16 = mybir.dt.bfloat16
ALU = mybir.AluOpType
ACTF = mybir.ActivationFunctionType

B, C, H, W = 32, 3, 512, 512
N = 256
NCORES = 8
BL = B // NCORES  # batches per core
KC = 4            # 128-row chunks of the 512-long axis
SMALL = 1e-4
DELTA_SCALE = (max(W, H) - 1) / (N - 1.0)


def _kernel_body(tc):
    nc = tc.nc
    x_d = nc.dram_tensor("x", [BL, C, H, W], F32, kind="ExternalInput").ap()
    p_d = nc.dram_tensor("p", [BL, 5], F32, kind="ExternalInput").ap()
    o_d = nc.dram_tensor("out", [BL, C, N, N], F32, kind="ExternalOutput").ap()

    with ExitStack() as ctx:
        consts = ctx.enter_context(tc.tile_pool(name="consts", bufs=1))
        params = ctx.enter_context(tc.tile_pool(name="params", bufs=1))
        xf32p = ctx.enter_context(tc.tile_pool(name="xf32p", bufs=3))
        xbfp = ctx.enter_context(tc.tile_pool(name="xbfp", bufs=3))
        tban = ctx.enter_context(tc.tile_pool(name="tban", bufs=10))
        bcp = ctx.enter_context(tc.tile_pool(name="bcp", bufs=2))
        dtmp = ctx.enter_context(tc.tile_pool(name="dtmp", bufs=3))
        sqtmp = ctx.enter_context(tc.tile_pool(name="sqtmp", bufs=3))
        fyxp = ctx.enter_context(tc.tile_pool(name="fyxp", bufs=5))
        outp = ctx.enter_context(tc.tile_pool(name="outp", bufs=4))
        rows = ctx.enter_context(tc.tile_pool(name="rows", bufs=4))
        colp = ctx.enter_context(tc.tile_pool(name="colp", bufs=2))
        invp = ctx.enter_context(tc.tile_pool(name="invp", bufs=2))
        # PSUM: 8 banks total — ps1 2 + ps2 2 + pscs 1 + pscol 1 + psbc 1
        # + psinvx 1
        ps1 = ctx.enter_context(tc.tile_pool(name="ps1", bufs=2, space="PSUM"))
        ps2 = ctx.enter_context(tc.tile_pool(name="ps2", bufs=2, space="PSUM"))
        pscs = ctx.enter_context(tc.tile_pool(name="pscs", bufs=1, space="PSUM"))
        pscol = ctx.enter_context(tc.tile_pool(name="pscol", bufs=1, space="PSUM"))
        psbc = ctx.enter_context(tc.tile_pool(name="psbc", bufs=1, space="PSUM"))
        psinvx = ctx.enter_context(tc.tile_pool(name="psinvx", bufs=1, space="PSUM"))

        # ---- constants -------------------------------------------------
        a_iota = consts.tile([128, 1], F32)  # partition index 0..127
        nc.gpsimd.iota(a_iota, pattern=[[0, 1]], base=0, channel_multiplier=1,
                       allow_small_or_imprecise_dtypes=True)
        iota4 = consts.tile([BL, 2 * N], F32)  # 0..255 twice, on BL partitions
        nc.gpsimd.iota(iota4, pattern=[[0, 2], [1, N]], base=0,
                       channel_multiplier=0, allow_small_or_imprecise_dtypes=True)
        ones_k = consts.tile([128, 1], BF16)  # colsum lhsT
        nc.vector.memset(ones_k, 1.0)
        one1 = consts.tile([1, 1], F32)       # row->col rhs
        nc.vector.memset(one1, 1.0)
        ones_r = consts.tile([1, 128], F32)   # broadcast lhsT (1 -> 128 parts)
        nc.vector.memset(ones_r, 1.0)
        small1 = consts.tile([1, 1], F32)     # filterbank-normalizer epsilon
        nc.vector.memset(small1, SMALL)

        # ---- per-batch attention params (partition = batch) ------------
        pt = params.tile([BL, 5], F32)
        nc.sync.dma_start(out=pt, in_=p_d)
        E = params.tile([BL, 3], F32)  # [sigma2, exp(p3), gamma]
        nc.scalar.activation(E, pt[:, 2:5], ACTF.Exp)
        delta = params.tile([BL, 1], F32)
        nc.vector.tensor_scalar(delta, E[:, 1:2], DELTA_SCALE, None, ALU.mult)
        g2 = params.tile([BL, 2], F32)  # [gx, gy]
        nc.vector.tensor_scalar(g2, pt[:, 0:2], W / 2.0, W / 2.0, ALU.mult, ALU.add)
        cyx = params.tile([BL, 2], F32)  # g - (N/2+0.5)*delta ; [:,0]=y uses gy
        nc.vector.scalar_tensor_tensor(cyx[:, 0:1], delta, -(N / 2.0 + 0.5),
                                       g2[:, 1:2], ALU.mult, ALU.add)
        nc.vector.scalar_tensor_tensor(cyx[:, 1:2], delta, -(N / 2.0 + 0.5),
                                       g2[:, 0:1], ALU.mult, ALU.add)
        # per-batch row: [mu_y (N) | mu_x (N) | nhs | gamma]
        M4 = params.tile([BL, 2 * N + 2], F32)
        nc.vector.tensor_scalar(M4[:, 0:N], iota4[:, 0:N], delta, cyx[:, 0:1],
                                ALU.mult, ALU.add)
        nc.vector.tensor_scalar(M4[:, N:2 * N], iota4[:, N:2 * N], delta,
                                cyx[:, 1:2], ALU.mult, ALU.add)
        nc.vector.reciprocal(M4[:, 2 * N:2 * N + 1], E[:, 0:1])
        nc.vector.tensor_scalar(M4[:, 2 * N:2 * N + 1], M4[:, 2 * N:2 * N + 1],
                                -0.5, None, ALU.mult)
        nc.vector.tensor_copy(M4[:, 2 * N + 1:2 * N + 2], E[:, 2:3])

        # all batches' rows flattened onto partition 0 (one sbuf->sbuf DMA)
        RW = 2 * N + 2
        stage = params.tile([1, BL * RW], F32)
        for sb_ in range(BL):
            nc.sync.dma_start(out=stage[:, sb_ * RW:(sb_ + 1) * RW],
                              in_=M4[sb_:sb_ + 1, :])

        for b in range(BL):
            # broadcast batch b's mu row + [nhs, gamma] to all 128 partitions
            r0 = b * RW
            ps_bc = psbc.tile([128, 2 * N], F32)
            nc.tensor.matmul(ps_bc, ones_r, stage[:, r0:r0 + 2 * N],
                             start=True, stop=True)
            bcmu = bcp.tile([128, 2 * N], F32)
            nc.scalar.copy(bcmu, ps_bc)
            ps_nhs = pscol.tile([128, 2], F32, tag="pcol")
            nc.tensor.matmul(ps_nhs, ones_r,
                             stage[:, r0 + 2 * N:r0 + 2 * N + 2],
                             start=True, stop=True)
            nhs_col = colp.tile([128, 2], F32)  # [:,0]=nhs  [:,1]=gamma
            nc.vector.tensor_copy(nhs_col, ps_nhs)

            # ---- filterbanks: Ty|Tx fused per chunk, unnormalized ------
            T = []
            invy_col = colp.tile([128, 2], F32)  # 1/(colsum_y+eps), n-major
            cs_ps = pscs.tile([1, 2 * N], F32)
            for k in range(KC):
                d_t = dtmp.tile([128, 2 * N], F32)
                # (mu - a_part) - 128k  (sign irrelevant after square)
                nc.vector.tensor_scalar(d_t, bcmu, a_iota, float(128 * k),
                                        ALU.subtract, ALU.subtract)
                sq_t = sqtmp.tile([128, 2 * N], F32)
                nc.scalar.activation(sq_t, d_t, ACTF.Square)
                T_t = tban.tile([128, 2 * N], BF16)
                nc.scalar.activation(T_t, sq_t, ACTF.Exp, scale=nhs_col[:, 0:1])
                T.append(T_t)
                nc.tensor.matmul(cs_ps, ones_k, T_t,
                                 start=(k == 0), stop=(k == KC - 1))
            # 1/(colsum + SMALL) = exp(-ln(colsum + SMALL)), both axes at once
            lnrow = rows.tile([1, 2 * N], F32)
            nc.scalar.activation(lnrow, cs_ps, ACTF.Ln, bias=small1[:, :])
            invrow = rows.tile([1, 2 * N], F32)
            nc.scalar.activation(invrow, lnrow, ACTF.Exp, scale=-1.0)
            # y-normalizer to column layout (n on partitions), * gamma
            for j in range(2):
                pcol = pscol.tile([128, 1], F32)
                nc.tensor.matmul(pcol, invrow[:, j * 128:(j + 1) * 128],
                                 one1, start=True, stop=True)
                nc.vector.tensor_scalar(invy_col[:, j:j + 1], pcol,
                                        nhs_col[:, 1:2], None, ALU.mult)
            # x-normalizer broadcast across partitions (m on free axis)
            invx_ps = psinvx.tile([128, N], F32)
            nc.tensor.matmul(invx_ps, ones_r, invrow[:, N:2 * N],
                             start=True, stop=True)
            invx_bc = invp.tile([128, N], F32)
            nc.scalar.copy(invx_bc, invx_ps)

            # ---- glimpse read: two chained GEMMs per channel -----------
            for c in range(C):
                xf = xf32p.tile([128, KC, W], F32)
                xt = xbfp.tile([128, KC, W], BF16)
                xsrc = x_d[b, c].rearrange("(hc p) w -> p hc w", p=128)
                for hc in range(KC):  # chunked so G1 starts as rows land
                    nc.sync.dma_start(out=xf[:, hc], in_=xsrc[:, hc])
                    nc.vector.tensor_copy(xt[:, hc], xf[:, hc])
                fyx = []
                for j in range(2):  # wc pairs
                    p1 = ps1.tile([128, 2 * N], F32)
                    for half in range(2):
                        wc = 2 * j + half
                        for hc in range(KC):
                            nc.tensor.matmul(
                                p1[:, half * N:(half + 1) * N],
                                xt[:, hc, wc * 128:(wc + 1) * 128],
                                T[hc][:, 0:N],
                                start=(hc == 0), stop=(hc == KC - 1))
                    f_t = fyxp.tile([128, 2 * N], BF16)
                    if j == 0:
                        nc.vector.tensor_copy(f_t, p1)
                    else:
                        nc.scalar.copy(f_t, p1)
                    fyx.append(f_t)
                ot = outp.tile([128, 2, N], F32)
                for nch in range(2):
                    p2 = ps2.tile([128, N], F32)
                    for wc in range(KC):
                        nc.tensor.matmul(
                            p2,
                            fyx[wc // 2][:, (wc % 2) * N + nch * 128:
                                         (wc % 2) * N + (nch + 1) * 128],
                            T[wc][:, N:2 * N],
                            start=(wc == 0), stop=(wc == KC - 1))
                    nc.vector.scalar_tensor_tensor(ot[:, nch, :], p2,
                                                   invy_col[:, nch:nch + 1],
                                                   invx_bc, ALU.mult, ALU.mult)
                nc.sync.dma_start(
                    out=o_d[b, c].rearrange("(nch p) m -> p nch m", p=128), in_=ot)


_NC_CACHE = None


def _build():
    global _NC_CACHE
    if _NC_CACHE is None:
        nc = bacc.Bacc("TRN2", target_bir_lowering=False, debug=False,
                       enable_asserts=False, num_devices=NCORES)
        with tile.TileContext(nc) as tc:
            _kernel_body(tc)
        # Steer bacc's greedy ACT table-set choice to the one set that has
        # Exp+Ln+Square+Copy+Identity, else every per-batch Ln costs two
        # ~2.7us table reloads. Only the selection input is patched — set
        # ids and on-device table contents are untouched.
        ours = {ACTF.Exp, ACTF.Ln, ACTF.Square, ACTF.Copy, ACTF.Identity}
        keep = "natural_log_exp_and_others"
        orig = bacc.get_activation_tables

        def steered(arch):
            return {k: (v if k == keep else set(v) - ours)
                    for k, v in orig(arch).items()}

        bacc.get_activation_tables = steered
        try:
            nc.compile()
        finally:
            bacc.get_activation_tables = orig
        _NC_CACHE = nc
    return _NC_CACHE


def _run(x, p, trace=False, **kw):
    nc = _build()
    x = np.ascontiguousarray(x, dtype=np.float32)
    p = np.ascontiguousarray(p, dtype=np.float32)
    assert x.shape == (B, C, H, W) and p.shape == (B, 5), (x.shape, p.shape)
    in_maps = [
        {"x": x[i * BL:(i + 1) * BL], "p": p[i * BL:(i + 1) * BL]}
        for i in range(NCORES)
    ]
    res = run_bass_kernel_spmd(nc, in_maps, list(range(NCORES)), trace=trace, **kw)
    out = np.concatenate([res.results[i]["out"] for i in range(NCORES)], axis=0)
    return out, res


def kernel(x, p):
    out, _ = _run(x, p)
    return out


# revision 41
# speedup vs baseline: 1.0291x; 1.0111x over previous
"""DifferentiableRAM (DRAW-style attention read) Trainium2 Bass kernel.

Reference computation (per batch b, channel c):
    gx = W*(p0+1)/2, gy = H*(p1+1)/2, sigma2 = exp(p2),
    delta = exp(p3)*(W-1)/(N-1), gamma = exp(p4)
    mu[i]  = g + delta*(i - N/2 - 0.5)                      i in [0,N)
    F[i,a] = exp(-(a-mu[i])^2 / (2 sigma2)) ;  Fn = F / (F.sum(a) + 1e-4)
    out[b,c] = gamma * Fy_n @ x[b,c] @ Fx_n^T                [N, N]

Strategy: pure data parallel over batch (B=32 -> 4 per core on 8 cores).
On-chip, both filterbanks are generated in *transposed* layout
T[a, i] = exp(-(a-mu_i)^2/(2s2)) (y and x side by side in one [128, 512]
tile per 128-row chunk) so both GEMMs contract along the partition axis
and the output lands in [n, m] row-major order:
    G1: FyxT[w, n] = sum_h x[h, w] * Ty[h, n]      (lhsT = x chunk)
    G2: raw[n, m]  = sum_w FyxT[w, n] * Tx[w, m]   (lhsT = FyxT chunk)
    out[n, m] = raw[n, m] * (gamma * invy[n]) * invx[m]
GEMMs run in bf16 (full PE rate; fp32 matmul is 1/4 rate, fp32r does not
survive walrus codegen).  Normalizers invy/invx = 1/(colsum + 1e-4) stay
fp32, computed as exp(-ln(colsum + 1e-4)) on ScalarE (ln+exp share one
LUT set) and applied to the fp32 PSUM of G2, so the final scaling is
full precision.
"""

import numpy as np
from contextlib import ExitStack

import concourse.tile as tile
from concourse import bacc, mybir
from concourse.bass_utils import run_bass_kernel_spmd

F32 = mybir.dt.float32
BF16 = mybir.dt.bfloat16
ALU = mybir.AluOpType
ACTF = mybir.ActivationFunctionType

B, C, H, W = 32, 3, 512, 512
N = 256
NCORES = 8
BL = B // NCORES  # batches per core
KC = 4            # 128-row chunks of the 512-long axis
SMALL = 1e-4
DELTA_SCALE = (max(W, H) - 1) / (N - 1.0)


def _kernel_body(tc):
    nc = tc.nc
    x_d = nc.dram_tensor("x", [BL, C, H, W], F32, kind="ExternalInput").ap()
    p_d = nc.dram_tensor("p", [BL, 5], F32, kind="ExternalInput").ap()
    o_d = nc.dram_tensor("out", [BL, C, N, N], F32, kind="ExternalOutput").ap()

    with ExitStack() as ctx:
        consts = ctx.enter_context(tc.tile_pool(name="consts", bufs=1))
        params = ctx.enter_context(tc.tile_pool(name="params", bufs=1))
        xf32p = ctx.enter_context(tc.tile_pool(name="xf32p", bufs=3))
        xbfp = ctx.enter_context(tc.tile_pool(name="xbfp", bufs=3))
        tban = ctx.enter_context(tc.tile_pool(name="tban", bufs=10))
        bcp = ctx.enter_context(tc.tile_pool(name="bcp", bufs=2))
        dtmp = ctx.enter_context(tc.tile_pool(name="dtmp", bufs=3))
        sqtmp = ctx.enter_context(tc.tile_pool(name="sqtmp", bufs=3))
        fyxp = ctx.enter_context(tc.tile_pool(name="fyxp", bufs=5))
        outp = ctx.enter_context(tc.tile_pool(name="outp", bufs=4))
        rows = ctx.enter_context(tc.tile_pool(name="rows", bufs=4))
        colp = ctx.enter_context(tc.tile_pool(name="colp", bufs=2))
        invp = ctx.enter_context(tc.tile_pool(name="invp", bufs=2))
        # PSUM: 8 banks total — ps1 2 + ps2 2 + pscs 1 + pscol 1 + psbc 1
        # + psinvx 1
        ps1 = ctx.enter_context(tc.tile_pool(name="ps1", bufs=2, space="PSUM"))
        ps2 = ctx.enter_context(tc.tile_pool(name="ps2", bufs=2, space="PSUM"))
        pscs = ctx.enter_context(tc.tile_pool(name="pscs", bufs=1, space="PSUM"))
        pscol = ctx.enter_context(tc.tile_pool(name="pscol", bufs=1, space="PSUM"))
        psbc = ctx.enter_context(tc.tile_pool(name="psbc", bufs=1, space="PSUM"))
        psinvx = ctx.enter_context(tc.tile_pool(name="psinvx", bufs=1, space="PSUM"))

        # ---- constants -------------------------------------------------
        a_iota = consts.tile([128, 1], F32)  # partition index 0..127
        nc.gpsimd.iota(a_iota, pattern=[[0, 1]], base=0, channel_multiplier=1,
                       allow_small_or_imprecise_dtypes=True)
        iota4 = consts.tile([BL, 2 * N], F32)  # 0..255 twice, on BL partitions
        nc.gpsimd.iota(iota4, pattern=[[0, 2], [1, N]], base=0,
                       channel_multiplier=0, allow_small_or_imprecise_dtypes=True)
        ones_k = consts.tile([128, 1], BF16)  # colsum lhsT
        nc.vector.memset(ones_k, 1.0)
        one1 = consts.tile([1, 1], F32)       # row->col rhs
        nc.vector.memset(one1, 1.0)
        ones_r = consts.tile([1, 128], F32)   # broadcast lhsT (1 -> 128 parts)
        nc.vector.memset(ones_r, 1.0)
        small1 = consts.tile([1, 1], F32)     # filterbank-normalizer epsilon
        nc.vector.memset(small1, SMALL)

        # ---- per-batch attention params (partition = batch) ------------
        pt = params.tile([BL, 5], F32)
        nc.sync.dma_start(out=pt, in_=p_d)
        E = params.tile([BL, 3], F32)  # [sigma2, exp(p3), gamma]
        nc.scalar.activation(E, pt[:, 2:5], ACTF.Exp)
        delta = params.tile([BL, 1], F32)
        nc.vector.tensor_scalar(delta, E[:, 1:2], DELTA_SCALE, None, ALU.mult)
        g2 = params.tile([BL, 2], F32)  # [gx, gy]
        nc.vector.tensor_scalar(g2, pt[:, 0:2], W / 2.0, W / 2.0, ALU.mult, ALU.add)
        cyx = params.tile([BL, 2], F32)  # g - (N/2+0.5)*delta ; [:,0]=y uses gy
        nc.vector.scalar_tensor_tensor(cyx[:, 0:1], delta, -(N / 2.0 + 0.5),
                                       g2[:, 1:2], ALU.mult, ALU.add)
        nc.vector.scalar_tensor_tensor(cyx[:, 1:2], delta, -(N / 2.0 + 0.5),
                                       g2[:, 0:1], ALU.mult, ALU.add)
        # per-batch row: [mu_y (N) | mu_x (N) | nhs | gamma]
        M4 = params.tile([BL, 2 * N + 2], F32)
        nc.vector.tensor_scalar(M4[:, 0:N], iota4[:, 0:N], delta, cyx[:, 0:1],
                                ALU.mult, ALU.add)
        nc.vector.tensor_scalar(M4[:, N:2 * N], iota4[:, N:2 * N], delta,
                                cyx[:, 1:2], ALU.mult, ALU.add)
        nc.vector.reciprocal(M4[:, 2 * N:2 * N + 1], E[:, 0:1])
        nc.vector.tensor_scalar(M4[:, 2 * N:2 * N + 1], M4[:, 2 * N:2 * N + 1],
                                -0.5, None, ALU.mult)
        nc.vector.tensor_copy(M4[:, 2 * N + 1:2 * N + 2], E[:, 2:3])

        # all batches' rows flattened onto partition 0 (one sbuf->sbuf DMA)
        RW = 2 * N + 2
        stage = params.tile([1, BL * RW], F32)
        for sb_ in range(BL):
            nc.sync.dma_start(out=stage[:, sb_ * RW:(sb_ + 1) * RW],
                              in_=M4[sb_:sb_ + 1, :])

        for b in range(BL):
            # broadcast batch b's mu row + [nhs, gamma] to all 128 partitions
            r0 = b * RW
            ps_bc = psbc.tile([128, 2 * N], F32)
            nc.tensor.matmul(ps_bc, ones_r, stage[:, r0:r0 + 2 * N],
                             start=True, stop=True)
            bcmu = bcp.tile([128, 2 * N], F32)
            nc.scalar.copy(bcmu, ps_bc)
            ps_nhs = pscol.tile([128, 2], F32, tag="pcol")
            nc.tensor.matmul(ps_nhs, ones_r,
                             stage[:, r0 + 2 * N:r0 + 2 * N + 2],
                             start=True, stop=True)
            nhs_col = colp.tile([128, 2], F32)  # [:,0]=nhs  [:,1]=gamma
            nc.vector.tensor_copy(nhs_col, ps_nhs)

            # ---- filterbanks: Ty|Tx fused per chunk, unnormalized ------
            T = []
            invy_col = colp.tile([128, 2], F32)  # 1/(colsum_y+eps), n-major
            cs_ps = pscs.tile([1, 2 * N], F32)
            for k in range(KC):
                d_t = dtmp.tile([128, 2 * N], F32)
                # (mu - a_part) - 128k  (sign irrelevant after square)
                nc.vector.tensor_scalar(d_t, bcmu, a_iota, float(128 * k),
                                        ALU.subtract, ALU.subtract)
                sq_t = sqtmp.tile([128, 2 * N], F32)
                nc.scalar.activation(sq_t, d_t, ACTF.Square)
                T_t = tban.tile([128, 2 * N], BF16)
                nc.scalar.activation(T_t, sq_t, ACTF.Exp, scale=nhs_col[:, 0:1])
                T.append(T_t)
                nc.tensor.matmul(cs_ps, ones_k, T_t,
                                 start=(k == 0), stop=(k == KC - 1))
            # 1/(colsum + SMALL) = exp(-ln(colsum + SMALL)), both axes at once
            lnrow = rows.tile([1, 2 * N], F32)
            nc.scalar.activation(lnrow, cs_ps, ACTF.Ln, bias=small1[:, :])
            invrow = rows.tile([1, 2 * N], F32)
            nc.scalar.activation(invrow, lnrow, ACTF.Exp, scale=-1.0)
            # y-normalizer to column layout (n on partitions), * gamma
            for j in range(2):
                pcol = pscol.tile([128, 1], F32)
                nc.tensor.matmul(pcol, invrow[:, j * 128:(j + 1) * 128],
                                 one1, start=True, stop=True)
                nc.vector.tensor_scalar(invy_col[:, j:j + 1], pcol,
                                        nhs_col[:, 1:2], None, ALU.mult)
            # x-normalizer broadcast across partitions (m on free axis)
            invx_ps = psinvx.tile([128, N], F32)
            nc.tensor.matmul(invx_ps, ones_r, invrow[:, N:2 * N],
                             start=True, stop=True)
            invx_bc = invp.tile([128, N], F32)
            nc.vector.tensor_copy(invx_bc, invx_ps)

            # ---- glimpse read: two chained GEMMs per channel -----------
            for c in range(C):
                xf = xf32p.tile([128, KC, W], F32)
                xt = xbfp.tile([128, KC, W], BF16)
                xsrc = x_d[b, c].rearrange("(hc p) w -> p hc w", p=128)
                for hc in range(KC):  # chunked so G1 starts as rows land
                    nc.sync.dma_start(out=xf[:, hc], in_=xsrc[:, hc])
                    nc.vector.tensor_copy(xt[:, hc], xf[:, hc])
                fyx = []
                for j in range(2):  # wc pairs
                    p1 = ps1.tile([128, 2 * N], F32)
                    for half in range(2):
                        wc = 2 * j + half
                        for hc in range(KC):
                            nc.tensor.matmul(
                                p1[:, half * N:(half + 1) * N],
                                xt[:, hc, wc * 128:(wc + 1) * 128],
                                T[hc][:, 0:N],
                                start=(hc == 0), stop=(hc == KC - 1))
                    f_t = fyxp.tile([128, 2 * N], BF16)
                    if j == 0:
                        nc.vector.tensor_copy(f_t, p1)
                    else:
                        nc.scalar.copy(f_t, p1)
                    fyx.append(f_t)
                ot = outp.tile([128, 2, N], F32)
                for nch in range(2):
                    p2 = ps2.tile([128, N], F32)
                    for wc in range(KC):
                        nc.tensor.matmul(
                            p2,
                            fyx[wc // 2][:, (wc % 2) * N + nch * 128:
                                         (wc % 2) * N + (nch + 1) * 128],
                            T[wc][:, N:2 * N],
                            start=(wc == 0), stop=(wc == KC - 1))
                    nc.vector.scalar_tensor_tensor(ot[:, nch, :], p2,
                                                   invy_col[:, nch:nch + 1],
                                                   invx_bc, ALU.mult, ALU.mult)
                nc.sync.dma_start(
                    out=o_d[b, c].rearrange("(nch p) m -> p nch m", p=128), in_=ot)


_NC_CACHE = None


def _build():
    global _NC_CACHE
    if _NC_CACHE is None:
        nc = bacc.Bacc("TRN2", target_bir_lowering=False, debug=False,
                       enable_asserts=False, num_devices=NCORES)
        with tile.TileContext(nc) as tc:
            _kernel_body(tc)
        # Steer bacc's greedy ACT table-set choice to the one set that has
        # Exp+Ln+Square+Copy+Identity, else every per-batch Ln costs two
        # ~2.7us table reloads. Only the selection input is patched — set
        # ids and on-device table contents are untouched.
        ours = {ACTF.Exp, ACTF.Ln, ACTF.Square, ACTF.Copy, ACTF.Identity}
        keep = "natural_log_exp_and_others"
        orig = bacc.get_activation_tables

        def steered(arch):
            return {k: (v if k == keep else set(v) - ours)
                    for k, v in orig(arch).items()}

        bacc.get_activation_tables = steered
        try:
            nc.compile()
        finally:
            bacc.get_activation_tables = orig
        _NC_CACHE = nc
    return _NC_CACHE


def _run(x, p, trace=False, **kw):
    nc = _build()
    x = np.ascontiguousarray(x, dtype=np.float32)
    p = np.ascontiguousarray(p, dtype=np.float32)
    assert x.shape == (B, C, H, W) and p.shape == (B, 5), (x.shape, p.shape)
    in_maps = [
        {"x": x[i * BL:(i + 1) * BL], "p": p[i * BL:(i + 1) * BL]}
        for i in range(NCORES)
    ]
    res = run_bass_kernel_spmd(nc, in_maps, list(range(NCORES)), trace=trace, **kw)
    out = np.concatenate([res.results[i]["out"] for i in range(NCORES)], axis=0)
    return out, res


def kernel(x, p):
    out, _ = _run(x, p)
    return out
